# revision 10
# baseline (speedup 1.0000x reference)
"""Trainium2 8-core kernel for nn_BAKT_QIKT (sparse causal attention with
top-k re-softmax).

Algorithm (validated numerically against the jax reference):
  scores s = q@k^T/sqrt(dk) with causal mask, p = softmax(s),
  rows >= k_index keep only top-k entries of p, re-softmax everything
  (masked entries contribute exp(0)=1 for rows < k_index), row 0 zeroed,
  out = w @ v.

Device mapping per (batch,head) pair (16 per core, data-parallel over batch):
  - scores in ~f32 precision via 3-term bf16-split matmul
    (qh@kh + ql@kh + qh@kl), causal mask added by an extra PE matmul
    (lhsT = strict-lower-tri * -1e38, rhs = identity) accumulating into the
    score PSUM.
  - e = exp(c*s) on ACT with row-sum accumulator Z (softmax denominators).
  - top-8 per row via DVE max8; kth = (k_index)-th largest of e.
  - second softmax shift-invariance: w~ = exp((e-kth)/Z) so kept entries
    are >= 1 and dropped < 1.  w2 = max(w~,1)-1 (kept-only, exact zeros),
    ind = (e >= kth) * 1/Z2 on GPSIMD; w2' = w2 * 1/Z2 on DVE.
    Z2 = sum(w2) + k_index (exactly k entries kept).
  - both layers PE-transposed (bf16) into separate PSUM tiles, merged by the
    mandatory PSUM->SBUF copy as tensor_tensor(add), then a single AV matmul
    chain accumulates out = (w2'+ind')^T-contracted with v into PSUM and is
    DMA'd straight to DRAM.
  Rows < k_index (and the zero-padded row 0) are computed on host - 0.5% of
  the work, removes all special cases from the device kernel.
"""
import os
import sys

sys.path.insert(0, "/opt/trn_rl_repo")

import numpy as np

BS, H, S, DK = 16, 8, 1024, 128
N_CORES = 8
B_PER_CORE = BS // N_CORES
BH = B_PER_CORE * H          # bh pairs per core
P = 128
NT = S // P                  # row tiles per bh
NEG = -1e32                  # reference's mask value
C_SCALE = 1.0 / np.sqrt(DK)

_CACHE = {}


def _np_bf16():
    import concourse.mybir as mybir
    return mybir.dt.np(mybir.dt.bfloat16)


def _build_nc(k_index: int, n_bh: int):
    import concourse.bass as bass
    import concourse.mybir as mybir
    from concourse.tile import TileContext

    BF = mybir.dt.bfloat16
    F32 = mybir.dt.float32
    AF = mybir.ActivationFunctionType
    OP = mybir.AluOpType

    debug = bool(os.environ.get("BAKT_DEBUG"))
    nc = bass.Bass()
    qh_ext = nc.declare_dram_parameter("qh", [n_bh, P, S], BF, isOutput=False)
    ql_ext = nc.declare_dram_parameter("ql", [n_bh, P, S], BF, isOutput=False)
    kh_ext = nc.declare_dram_parameter("kh", [n_bh, P, S], BF, isOutput=False)
    kl_ext = nc.declare_dram_parameter("kl", [n_bh, P, S], BF, isOutput=False)
    v_ext = nc.declare_dram_parameter("v", [n_bh, S, DK], BF, isOutput=False)
    tri_ext = nc.declare_dram_parameter("tri", [P, P], BF, isOutput=False)
    id_ext = nc.declare_dram_parameter("ident", [P, P], BF, isOutput=False)
    out_ext = nc.declare_dram_parameter("out", [n_bh, S, DK], F32, isOutput=True)
    if debug:
        e_dbg = nc.declare_dram_parameter("e_dbg", [P, 4608], F32, isOutput=True)
        m8_dbg = nc.declare_dram_parameter("m8_dbg", [P, NT, 8], F32, isOutput=True)
        zp_dbg = nc.declare_dram_parameter("zp_dbg", [P, NT], F32, isOutput=True)
        w2_dbg = nc.declare_dram_parameter("w2_dbg", [P, 4608], F32, isOutput=True)
        ws_dbg = nc.declare_dram_parameter("ws_dbg", [P, S], F32, isOutput=True)
        wt_dbg = nc.declare_dram_parameter("wt_dbg", [P, S], F32, isOutput=True)
        z2_dbg = nc.declare_dram_parameter("z2_dbg", [P, NT], F32, isOutput=True)

    with TileContext(nc) as tc:
        with (
            tc.tile_pool(name="consts", bufs=1) as cp,
            tc.tile_pool(name="qk", bufs=2) as qkp,
            tc.tile_pool(name="vp", bufs=2) as vp,
            tc.tile_pool(name="ep", bufs=2) as ep,
            tc.tile_pool(name="wp", bufs=2) as wp,
            tc.tile_pool(name="small", bufs=2) as sp,
            tc.tile_pool(name="ps_s", bufs=2, space="PSUM") as ps_s,
            tc.tile_pool(name="ps_t", bufs=1, space="PSUM") as ps_t,
            tc.tile_pool(name="ps_o", bufs=2, space="PSUM") as ps_o,
        ):
            tri = cp.tile([P, P], BF, tag="tri")
            nc.sync.dma_start(out=tri[:], in_=tri_ext[:])
            ident = cp.tile([P, P], BF, tag="ident")
            nc.sync.dma_start(out=ident[:], in_=id_ext[:])

            for bh in range(n_bh):
                qh = qkp.tile([P, S], BF, tag="qh")
                nc.sync.dma_start(out=qh[:], in_=qh_ext[bh])
                ql = qkp.tile([P, S], BF, tag="ql")
                nc.sync.dma_start(out=ql[:], in_=ql_ext[bh])
                kh = qkp.tile([P, S], BF, tag="kh")
                nc.sync.dma_start(out=kh[:], in_=kh_ext[bh])
                kl = qkp.tile([P, S], BF, tag="kl")
                nc.sync.dma_start(out=kl[:], in_=kl_ext[bh])
                v = vp.tile([P, NT, DK], BF, tag="v")
                nc.sync.dma_start(
                    out=v[:], in_=v_ext[bh].rearrange("(c p) d -> p c d", p=P)
                )

                e = ep.tile([P, 4608], F32, tag="e")
                m8p = sp.tile([P, NT, 8], F32, tag="m8p")
                zp = sp.tile([P, NT], F32, tag="zp")

                offs = [64 * t * (t + 1) for t in range(NT + 1)]

                # ---- phase A: scores, exp, row stats ----
                for t in range(NT):
                    C = P * (t + 1)
                    s_ps = ps_s.tile([P, S], F32, tag="s", space="PSUM")
                    qslice_h = qh[:, t * P:(t + 1) * P]
                    qslice_l = ql[:, t * P:(t + 1) * P]
                    for j0 in range(0, C, 512):
                        j1 = min(j0 + 512, C)
                        last = j1 == C
                        nc.tensor.matmul(s_ps[:, j0:j1], qslice_h, kh[:, j0:j1],
                                         start=True, stop=False)
                        nc.tensor.matmul(s_ps[:, j0:j1], qslice_l, kh[:, j0:j1],
                                         start=False, stop=False)
                        nc.tensor.matmul(s_ps[:, j0:j1], qslice_h, kl[:, j0:j1],
                                         start=False, stop=not last,
                                         skip_group_check=True)
                    # causal mask on the diagonal block via PE accumulation
                    nc.tensor.matmul(s_ps[:, t * P:C], tri[:], ident[:],
                                     start=False, stop=True,
                                     skip_group_check=True)
                    esl = e[:, offs[t]:offs[t] + C]
                    nc.scalar.activation(out=esl, in_=s_ps[:, :C], func=AF.Exp,
                                         scale=C_SCALE,
                                         accum_out=zp[:, t:t + 1])
                    nc.vector.max(out=m8p[:, t, :], in_=esl)

                # ---- packed per-bh stats ----
                rz = sp.tile([P, NT], F32, tag="rz")
                nc.vector.reciprocal(rz[:], zp[:])
                nkz = sp.tile([P, NT], F32, tag="nkz")
                nc.vector.tensor_tensor(out=nkz[:], in0=m8p[:, :, k_index - 1],
                                        in1=rz[:], op=OP.mult)
                nc.vector.tensor_scalar(nkz[:], nkz[:], -1.0, None, op0=OP.mult)

                # ---- phase B: w~ and w2 ----
                wtb = wp.tile([P, 4608], BF, tag="wtb")
                w2 = wp.tile([P, 4608], BF, tag="w2")
                z2a = sp.tile([P, NT], F32, tag="z2a")
                for t in range(NT):
                    C = P * (t + 1)
                    esl = e[:, offs[t]:offs[t] + C]
                    nc.scalar.activation(out=wtb[:, offs[t]:offs[t] + C],
                                         in_=esl, func=AF.Exp,
                                         scale=rz[:, t:t + 1],
                                         bias=nkz[:, t:t + 1])
                    # w2 = max(w~,1)-1 needs two passes: with accum_out
                    # present, op1 is the accumulation op, not elementwise.
                    nc.vector.tensor_scalar(w2[:, offs[t]:offs[t] + C],
                                            wtb[:, offs[t]:offs[t] + C],
                                            1.0, None, op0=OP.max)
                    nc.vector.tensor_scalar(w2[:, offs[t]:offs[t] + C],
                                            w2[:, offs[t]:offs[t] + C],
                                            1.0, 0.0, op0=OP.subtract,
                                            op1=OP.add,
                                            accum_out=z2a[:, t:t + 1])

                z2 = sp.tile([P, NT], F32, tag="z2")
                nc.vector.tensor_scalar(z2[:], z2a[:], float(k_index), None,
                                        op0=OP.add)
                rz2 = sp.tile([P, NT], F32, tag="rz2")
                nc.vector.reciprocal(rz2[:], z2[:])

                if debug and bh == 0:
                    nc.sync.dma_start(out=e_dbg[:], in_=e[:])
                    nc.sync.dma_start(out=m8_dbg[:], in_=m8p[:])
                    nc.sync.dma_start(out=zp_dbg[:], in_=zp[:])
                    nc.sync.dma_start(out=z2_dbg[:], in_=z2[:])
                    w2f = ep.tile([P, 4608], F32, tag="w2f")
                    nc.vector.tensor_copy(w2f[:], w2[:])
                    nc.sync.dma_start(out=w2_dbg[:], in_=w2f[:])

                # ---- phase C: scatter-free masked weights, transpose, AV ----
                for t in range(NT):
                    C = P * (t + 1)
                    esl = e[:, offs[t]:offs[t] + C]
                    ind = wp.tile([P, S], BF, tag="ind")
                    nc.gpsimd.tensor_scalar(ind[:, :C], esl,
                                            m8p[:, t, k_index - 1:k_index],
                                            None, op0=OP.is_ge)
                    wsum = wp.tile([P, S], BF, tag="wsum")
                    nc.vector.tensor_tensor(out=wsum[:, :C],
                                            in0=w2[:, offs[t]:offs[t] + C],
                                            in1=ind[:, :C], op=OP.add)
                    wt_ps = ps_t.tile([P, S], BF, tag="wt", space="PSUM")
                    for c in range(t + 1):
                        csl = slice(c * P, (c + 1) * P)
                        nc.tensor.matmul(wt_ps[:, csl], wsum[:, csl], ident[:],
                                         is_transpose=True)
                    wt_sb = wp.tile([P, S], BF, tag="wt_sb")
                    nc.vector.tensor_copy(wt_sb[:, :C], wt_ps[:, :C])
                    if debug and bh == 0 and t == 2:
                        wsf = wp.tile([P, S], F32, tag="wsf")
                        nc.vector.tensor_copy(wsf[:, :C], wsum[:, :C])
                        nc.sync.dma_start(out=ws_dbg[:, :C], in_=wsf[:, :C])
                        wtf = wp.tile([P, S], F32, tag="wtf")
                        nc.vector.tensor_copy(wtf[:, :C], wt_sb[:, :C])
                        nc.sync.dma_start(out=wt_dbg[:, :C], in_=wtf[:, :C])
                    av_ps = ps_o.tile([P, DK], F32, tag="av", space="PSUM")
                    for c in range(t + 1):
                        csl = slice(c * P, (c + 1) * P)
                        nc.tensor.matmul(av_ps[:], wt_sb[:, csl], v[:, c, :],
                                         start=(c == 0), stop=(c == t))
                    out_sb = wp.tile([P, DK], F32, tag="out_sb")
                    nc.vector.tensor_scalar(out_sb[:], av_ps[:],
                                            rz2[:, t:t + 1], None, op0=OP.mult)
                    nc.sync.dma_start(out=out_ext[bh, t * P:(t + 1) * P, :],
                                      in_=out_sb[:])
    return nc


def _get_runner(k_index: int, n_bh: int):
    """Build + jit once; reuse across calls (compile is minutes)."""
    key = (k_index, n_bh)
    if key in _CACHE:
        return _CACHE[key]

    import birfix_inline  # noqa: F401  (installed below; kept for clarity)

    nc = _build_nc(k_index, n_bh)

    import jax
    import numpy as _np
    from jax.sharding import Mesh, PartitionSpec
    from jax.experimental.shard_map import shard_map
    import concourse.mybir as mybir
    from concourse import bass2jax
    from concourse.bass2jax import _bass_exec_p, partition_id_tensor

    bass2jax.install_neuronx_cc_hook()

    partition_name = (nc.partition_id_tensor.name
                      if nc.partition_id_tensor else None)
    in_names, out_names, out_avals, zero_outs = [], [], [], []
    for alloc in nc.m.functions[0].allocations:
        if not isinstance(alloc, mybir.MemoryLocationSet):
            continue
        name = alloc.memorylocations[0].name
        if alloc.kind == "ExternalInput":
            if name != partition_name:
                in_names.append(name)
        elif alloc.kind == "ExternalOutput":
            shape = tuple(alloc.tensor_shape)
            dtype = mybir.dt.np(alloc.dtype)
            out_names.append(name)
            out_avals.append(jax.core.ShapedArray(shape, dtype))
            zero_outs.append(_np.zeros(shape, dtype))
    n_params = len(in_names)
    n_outs = len(out_avals)
    in_names_all = list(in_names) + list(out_names)
    if partition_name is not None:
        in_names_all.append(partition_name)

    def _body(*args):
        operands = list(args)
        if partition_name is not None:
            operands.append(partition_id_tensor())
        outs = _bass_exec_p.bind(
            *operands,
            out_avals=tuple(out_avals),
            in_names=tuple(in_names_all),
            out_names=tuple(out_names),
            lowering_input_output_aliases=(),
            sim_require_finite=True,
            sim_require_nnan=True,
            nc=nc,
        )
        return tuple(outs)

    devices = jax.devices()[:N_CORES]
    mesh = Mesh(np.asarray(devices), ("core",))
    in_specs = (PartitionSpec("core"),) * (n_params + n_outs)
    out_specs = (PartitionSpec("core"),) * n_outs
    donate = tuple(range(n_params, n_params + n_outs))
    sharded = jax.jit(
        shard_map(_body, mesh=mesh, in_specs=in_specs, out_specs=out_specs,
                  check_rep=False),
        donate_argnums=donate, keep_unused=True,
    )

    runner = {
        "sharded": sharded,
        "in_names": in_names,
        "out_names": out_names,
        "out_avals": out_avals,
        "zero_outs": zero_outs,
        "nc": nc,
    }
    _CACHE[key] = runner
    return runner


def _host_prep(q, k, v):
    """Shard + transform inputs for all cores. Returns dict name->global
    (n_cores*dim0, ...) arrays for shard_map."""
    bf16 = _np_bf16()
    qf = np.ascontiguousarray(q.reshape(BS * H, S, DK).transpose(0, 2, 1),
                              dtype=np.float32)
    kf = np.ascontiguousarray(k.reshape(BS * H, S, DK).transpose(0, 2, 1),
                              dtype=np.float32)
    qh = qf.astype(bf16)
    ql = (qf - qh.astype(np.float32)).astype(bf16)
    kh = kf.astype(bf16)
    kl = (kf - kh.astype(np.float32)).astype(bf16)
    vb = v.reshape(BS * H, S, DK).astype(bf16)

    tri = np.where(np.arange(P)[:, None] > np.arange(P)[None, :],
                   np.float32(-1e38), np.float32(0.0)).astype(bf16)
    ident = np.eye(P, dtype=np.float32).astype(bf16)

    # global arrays: concat per-core shards along axis 0.
    # core c handles bh-flat rows [c*BH : (c+1)*BH] already (batch-major).
    glob = {
        "qh": qh, "ql": ql, "kh": kh, "kl": kl, "v": vb,
        "tri": np.concatenate([tri[None]] * N_CORES, 0).reshape(
            N_CORES * P, P),
        "ident": np.concatenate([ident[None]] * N_CORES, 0).reshape(
            N_CORES * P, P),
    }
    return glob


def _host_rows(q, k, v, k_index):
    """Exact reference math for rows 0..k_index-1 (row 0 is zero-padded)."""
    qq = q[:, :, :k_index, :].astype(np.float64)
    kk = k.astype(np.float64)
    vv = v.astype(np.float64)
    s = np.einsum("bhqd,bhkd->bhqk", qq, kk) / np.sqrt(DK)
    j = np.arange(S)[None, None, None, :]
    i = np.arange(k_index)[None, None, :, None]
    s = np.where(j > i, NEG, s)
    p = np.exp(s - s.max(-1, keepdims=True))
    p = p / p.sum(-1, keepdims=True)
    w = np.exp(p)
    w = w / w.sum(-1, keepdims=True)
    out = np.einsum("bhqk,bhkd->bhqd", w, vv).astype(np.float32)
    out[:, :, 0, :] = 0.0
    return out


def _fallback(q, k, v, mask, k_index):
    """Pure-numpy replica of the reference (arbitrary mask / k_index)."""
    q64, k64, v64 = (x.astype(np.float64) for x in (q, k, v))
    s = np.einsum("bhqd,bhkd->bhqk", q64, k64) / np.sqrt(DK)
    s = np.where(np.asarray(mask) == 0, NEG, s)
    p = np.exp(s - s.max(-1, keepdims=True))
    p = p / p.sum(-1, keepdims=True)
    pa = p[:, :, :k_index, :]
    pb = p[:, :, k_index:, :]
    kth = -np.sort(-pb, axis=-1)[..., k_index - 1:k_index]
    pb = np.where(pb - kth >= 0, pb, NEG)
    sc = np.concatenate([pa, pb], axis=2)
    w = np.exp(sc - sc.max(-1, keepdims=True))
    w = w / w.sum(-1, keepdims=True)
    w[:, :, 0, :] = 0.0
    out = np.einsum("bhqk,bhkd->bhqd", w, v64)
    return out.astype(np.float32)


def kernel(q, k, v, mask, k_index):
    import birfix
    birfix.install()

    q = np.asarray(q, dtype=np.float32)
    k = np.asarray(k, dtype=np.float32)
    v = np.asarray(v, dtype=np.float32)
    ki = int(np.asarray(k_index))

    mask_np = np.asarray(mask)
    tril_ok = bool(
        (mask_np.reshape(S, S) == np.tril(np.ones((S, S), mask_np.dtype))).all()
    )
    if not tril_ok or not (1 <= ki <= 8):
        return _fallback(q, k, v, mask, ki)

    runner = _get_runner(ki, BH)
    glob = _host_prep(q, k, v)
    args = [glob[n] for n in runner["in_names"]]
    zeros = [np.zeros((N_CORES * z.shape[0], *z.shape[1:]), z.dtype)
             for z in runner["zero_outs"]]
    outs = runner["sharded"](*args, *zeros)
    out = np.array(outs[runner["out_names"].index("out")])
    out = out.reshape(BS, H, S, DK)
    out[:, :, :ki, :] = _host_rows(q, k, v, ki)
    return out


# birfix must be importable when kernel.py is standalone: embed a copy.
try:
    import birfix  # noqa: F401
except ImportError:
    import types

    _birfix_src = '''
import json
LIMIT = 1
_PATCHED = False

def split_waits_json(bir_json):
    d = json.loads(bir_json)
    cnt = 0
    for f in d.get("functions", []):
        for b in f.get("blocks", []):
            il = b.get("instructions")
            if not il:
                continue
            out = []
            changed = False
            for i in il:
                si = i.get("sync_info")
                waits = (si or {}).get("on_wait") or []
                if len(waits) > LIMIT:
                    changed = True
                    head, rest = waits[:-LIMIT], waits[-LIMIT:]
                    for ci in range(0, len(head), LIMIT):
                        cnt += 1
                        out.append({
                            "debug": i.get("debug", 0),
                            "engine": i["engine"],
                            "ins": [],
                            "is_reset_sema": False,
                            "name": "I-wsplit-%d" % cnt,
                            "opcode": "Drain",
                            "outs": [],
                            "sync_info": {"on_update": [],
                                          "on_wait": head[ci:ci + LIMIT]},
                        })
                    si["on_wait"] = rest
                out.append(i)
            if changed:
                b["instructions"] = out
    return json.dumps(d).encode()

def install():
    global _PATCHED
    if _PATCHED:
        return
    import concourse.bass2jax as b2j
    import concourse.bass_utils as bu
    orig = bu.compile_bir_kernel
    def patched(bir_json, tmpdir, neff_name="file.neff"):
        return orig(split_waits_json(bir_json), tmpdir, neff_name=neff_name)
    b2j.compile_bir_kernel = patched
    bu.compile_bir_kernel = patched
    _PATCHED = True
'''
    birfix = types.ModuleType("birfix")
    exec(_birfix_src, birfix.__dict__)
    sys.modules["birfix"] = birfix

sys.modules.setdefault("birfix_inline", sys.modules.get("birfix"))


# revision 19
# speedup vs baseline: 1.2617x; 1.2617x over previous
"""Trainium2 8-core kernel for nn_BAKT_QIKT (sparse causal attention with
top-k re-softmax).

Algorithm (validated numerically against the jax reference):
  scores s = q@k^T/sqrt(dk) with causal mask, p = softmax(s),
  rows >= k_index keep only top-k entries of p, re-softmax everything
  (masked entries contribute exp(0)=1 for rows < k_index), row 0 zeroed,
  out = w @ v.

Device mapping per (batch,head) pair (16 per core, data-parallel over batch):
  - scores in ~f32 precision via 3-term bf16-split matmul
    (qh@kh + ql@kh + qh@kl), causal mask added by an extra PE matmul
    (lhsT = strict-lower-tri * -1e38, rhs = identity) accumulating into the
    score PSUM.
  - e = exp(c*s) on ACT with row-sum accumulator Z (softmax denominators).
  - top-8 per row via DVE max8; kth = (k_index)-th largest of e.
  - second softmax shift-invariance: w~ = exp((e-kth)/Z) so kept entries
    are >= 1 and dropped < 1.  w2 = max(w~,1)-1 (kept-only, exact zeros),
    ind = (e >= kth) * 1/Z2 on GPSIMD; w2' = w2 * 1/Z2 on DVE.
    Z2 = sum(w2) + k_index (exactly k entries kept).
  - both layers PE-transposed (bf16) into separate PSUM tiles, merged by the
    mandatory PSUM->SBUF copy as tensor_tensor(add), then a single AV matmul
    chain accumulates out = (w2'+ind')^T-contracted with v into PSUM and is
    DMA'd straight to DRAM.
  Rows < k_index (and the zero-padded row 0) are computed on host - 0.5% of
  the work, removes all special cases from the device kernel.
"""
import os
import sys

sys.path.insert(0, "/opt/trn_rl_repo")

import numpy as np

BS, H, S, DK = 16, 8, 1024, 128
N_CORES = 8
B_PER_CORE = BS // N_CORES
BH = B_PER_CORE * H          # bh pairs per core
P = 128
NT = S // P                  # row tiles per bh
NEG = -1e32                  # reference's mask value
C_SCALE = 1.0 / np.sqrt(DK)

_CACHE = {}


def _np_bf16():
    import concourse.mybir as mybir
    return mybir.dt.np(mybir.dt.bfloat16)


def _build_nc(k_index: int, n_bh: int):
    import concourse.bass as bass
    import concourse.mybir as mybir
    from concourse.tile import TileContext

    BF = mybir.dt.bfloat16
    F32 = mybir.dt.float32
    AF = mybir.ActivationFunctionType
    OP = mybir.AluOpType

    debug = bool(os.environ.get("BAKT_DEBUG"))
    nc = bass.Bass()
    qk4_ext = nc.declare_dram_parameter("qk4", [n_bh, 4, P, S], BF,
                                        isOutput=False)
    v_ext = nc.declare_dram_parameter("v", [n_bh, S, DK], BF, isOutput=False)
    tri_ext = nc.declare_dram_parameter("tri", [P, P], BF, isOutput=False)
    id_ext = nc.declare_dram_parameter("ident", [P, P], BF, isOutput=False)
    out_ext = nc.declare_dram_parameter("out", [n_bh, S, DK], F32, isOutput=True)
    if debug:
        e_dbg = nc.declare_dram_parameter("e_dbg", [P, 4608], F32, isOutput=True)
        m8_dbg = nc.declare_dram_parameter("m8_dbg", [P, NT, 8], F32, isOutput=True)
        zp_dbg = nc.declare_dram_parameter("zp_dbg", [P, NT], F32, isOutput=True)
        w2_dbg = nc.declare_dram_parameter("w2_dbg", [P, 4608], F32, isOutput=True)
        ws_dbg = nc.declare_dram_parameter("ws_dbg", [P, S], F32, isOutput=True)
        wt_dbg = nc.declare_dram_parameter("wt_dbg", [P, S], F32, isOutput=True)
        z2_dbg = nc.declare_dram_parameter("z2_dbg", [P, NT], F32, isOutput=True)

    with TileContext(nc) as tc:
        with (
            tc.tile_pool(name="consts", bufs=1) as cp,
            tc.tile_pool(name="qk", bufs=2) as qkp,
            tc.tile_pool(name="vp", bufs=2) as vp,
            tc.tile_pool(name="ep", bufs=2) as ep,
            tc.tile_pool(name="wp", bufs=2) as wp,
            tc.tile_pool(name="small", bufs=2) as sp,
            tc.tile_pool(name="ps_s", bufs=2, space="PSUM") as ps_s,
            tc.tile_pool(name="ps_t", bufs=1, space="PSUM") as ps_t,
            tc.tile_pool(name="ps_o", bufs=2, space="PSUM") as ps_o,
        ):
            tri = cp.tile([P, P], BF, tag="tri")
            nc.sync.dma_start(out=tri[:], in_=tri_ext[:])
            ident = cp.tile([P, P], BF, tag="ident")
            nc.sync.dma_start(out=ident[:], in_=id_ext[:])
            # kmc[0, t] = k_index - C_t  (Z2 = sum(max(w~,1)) + k - C)
            kmc = cp.tile([P, NT], F32, tag="kmc")
            for t in range(NT):
                nc.vector.memset(kmc[:, t:t + 1], float(k_index - P * (t + 1)))

            for bh in range(n_bh):
                qk4 = qkp.tile([P, 4, S], BF, tag="qk4")
                nc.sync.dma_start(
                    out=qk4[:], in_=qk4_ext[bh].rearrange("f p s -> p f s"))
                qh = qk4[:, 0, :]
                ql = qk4[:, 1, :]
                kh = qk4[:, 2, :]
                kl = qk4[:, 3, :]
                v = vp.tile([P, NT, DK], BF, tag="v")
                nc.sync.dma_start(
                    out=v[:], in_=v_ext[bh].rearrange("(c p) d -> p c d", p=P)
                )

                e = ep.tile([P, 4608], F32, tag="e")
                m8p = sp.tile([P, NT, 8], F32, tag="m8p")
                zp = sp.tile([P, NT], F32, tag="zp")

                offs = [64 * t * (t + 1) for t in range(NT + 1)]

                # ---- phase A: scores, exp, row stats ----
                for t in range(NT):
                    C = P * (t + 1)
                    s_ps = ps_s.tile([P, S], F32, tag="s", space="PSUM")
                    qslice_h = qh[:, t * P:(t + 1) * P]
                    qslice_l = ql[:, t * P:(t + 1) * P]
                    for j0 in range(0, C, 512):
                        j1 = min(j0 + 512, C)
                        last = j1 == C
                        nc.tensor.matmul(s_ps[:, j0:j1], qslice_h, kh[:, j0:j1],
                                         start=True, stop=False)
                        nc.tensor.matmul(s_ps[:, j0:j1], qslice_l, kh[:, j0:j1],
                                         start=False, stop=False)
                        nc.tensor.matmul(s_ps[:, j0:j1], qslice_h, kl[:, j0:j1],
                                         start=False, stop=not last,
                                         skip_group_check=True)
                    # causal mask on the diagonal block via PE accumulation
                    nc.tensor.matmul(s_ps[:, t * P:C], tri[:], ident[:],
                                     start=False, stop=True,
                                     skip_group_check=True)
                    esl = e[:, offs[t]:offs[t] + C]
                    nc.scalar.activation(out=esl, in_=s_ps[:, :C], func=AF.Exp,
                                         scale=C_SCALE,
                                         accum_out=zp[:, t:t + 1])
                    nc.vector.max(out=m8p[:, t, :], in_=esl)

                # ---- packed per-bh stats ----
                rz = sp.tile([P, NT], F32, tag="rz")
                nc.vector.reciprocal(rz[:], zp[:])
                nkz = sp.tile([P, NT], F32, tag="nkz")
                nc.vector.tensor_tensor(out=nkz[:], in0=m8p[:, :, k_index - 1],
                                        in1=rz[:], op=OP.mult)
                nc.vector.tensor_scalar(nkz[:], nkz[:], -1.0, None, op0=OP.mult)

                # ---- phase B: w~ = exp((e-kth)/Z), wmax = max(w~,1) ----
                wtb = wp.tile([P, 4608], BF, tag="wtb")
                wmax = wp.tile([P, 4608], BF, tag="wmax")
                z2a = sp.tile([P, NT], F32, tag="z2a")
                for t in range(NT):
                    C = P * (t + 1)
                    esl = e[:, offs[t]:offs[t] + C]
                    nc.scalar.activation(out=wtb[:, offs[t]:offs[t] + C],
                                         in_=esl, func=AF.Exp,
                                         scale=rz[:, t:t + 1],
                                         bias=nkz[:, t:t + 1])
                    nc.vector.tensor_scalar(wmax[:, offs[t]:offs[t] + C],
                                            wtb[:, offs[t]:offs[t] + C],
                                            1.0, 0.0, op0=OP.max, op1=OP.add,
                                            accum_out=z2a[:, t:t + 1])

                # Z2 = sum(max(w~,1)) - C + k  per tile
                z2 = sp.tile([P, NT], F32, tag="z2")
                nc.vector.tensor_tensor(out=z2[:], in0=z2a[:],
                                        in1=kmc[:],
                                        op=OP.add)
                rz2 = sp.tile([P, NT], F32, tag="rz2")
                nc.vector.reciprocal(rz2[:], z2[:])

                if debug and bh == 0:
                    nc.sync.dma_start(out=e_dbg[:], in_=e[:])
                    nc.sync.dma_start(out=m8_dbg[:], in_=m8p[:])
                    nc.sync.dma_start(out=zp_dbg[:], in_=zp[:])
                    w2f = ep.tile([P, 4608], F32, tag="w2f")
                    nc.vector.tensor_copy(w2f[:], wmax[:])
                    nc.sync.dma_start(out=w2_dbg[:], in_=w2f[:])

                # ---- phase C: masked weights, Z2, transpose, AV ----
                for t in range(NT):
                    C = P * (t + 1)
                    esl = e[:, offs[t]:offs[t] + C]
                    # ind' = (e >= kth) - 1  in {0, -1}
                    ind = wp.tile([P, S], BF, tag="ind")
                    nc.gpsimd.tensor_scalar(ind[:, :C], esl,
                                            m8p[:, t, k_index - 1:k_index],
                                            -1.0, op0=OP.is_ge, op1=OP.add)
                    # wsum = wmax + ind' = {w~ kept, 0 dropped}
                    wsum = wp.tile([P, S], BF, tag="wsum")
                    nc.vector.tensor_tensor(out=wsum[:, :C],
                                            in0=wmax[:, offs[t]:offs[t] + C],
                                            in1=ind[:, :C], op=OP.add)
                    wt_ps = ps_t.tile([P, S], BF, tag="wt", space="PSUM")
                    for c in range(t + 1):
                        csl = slice(c * P, (c + 1) * P)
                        nc.tensor.matmul(wt_ps[:, csl], wsum[:, csl], ident[:],
                                         is_transpose=True)
                    wt_sb = wp.tile([P, S], BF, tag="wt_sb")
                    nc.vector.tensor_copy(wt_sb[:, :C], wt_ps[:, :C])
                    if debug and bh == 0 and t == 2:
                        wsf = wp.tile([P, S], F32, tag="wsf")
                        nc.vector.tensor_copy(wsf[:, :C], wsum[:, :C])
                        nc.sync.dma_start(out=ws_dbg[:, :C], in_=wsf[:, :C])
                        wtf = wp.tile([P, S], F32, tag="wtf")
                        nc.vector.tensor_copy(wtf[:, :C], wt_sb[:, :C])
                        nc.sync.dma_start(out=wt_dbg[:, :C], in_=wtf[:, :C])
                    if t % 4 == 0:
                        av_ps = ps_o.tile([P, 4 * DK], F32, tag="av",
                                          space="PSUM")
                    tt = t % 4
                    for c in range(t + 1):
                        csl = slice(c * P, (c + 1) * P)
                        nc.tensor.matmul(av_ps[:, tt * DK:(tt + 1) * DK],
                                         wt_sb[:, csl], v[:, c, :],
                                         start=(c == 0), stop=(c == t))
                    if t % 4 == 3:
                        t0 = t - 3
                        out4 = wp.tile([P, 4, DK], F32, tag="out4")
                        nc.vector.tensor_tensor(
                            out=out4[:],
                            in0=av_ps[:].rearrange("p (c d) -> p c d", c=4),
                            in1=rz2[:, t0:t0 + 4, None].to_broadcast(
                                [P, 4, DK]),
                            op=OP.mult)
                        nc.sync.dma_start(
                            out=out_ext[bh, t0 * P:(t0 + 4) * P, :].rearrange(
                                "(c p) d -> p c d", p=P),
                            in_=out4[:])
    return nc


def _get_runner(k_index: int, n_bh: int):
    """Build + jit once; reuse across calls (compile is minutes)."""
    key = (k_index, n_bh)
    if key in _CACHE:
        return _CACHE[key]

    import birfix_inline  # noqa: F401  (installed below; kept for clarity)

    nc = _build_nc(k_index, n_bh)

    import jax
    import numpy as _np
    from jax.sharding import Mesh, PartitionSpec
    from jax.experimental.shard_map import shard_map
    import concourse.mybir as mybir
    from concourse import bass2jax
    from concourse.bass2jax import _bass_exec_p, partition_id_tensor

    bass2jax.install_neuronx_cc_hook()

    partition_name = (nc.partition_id_tensor.name
                      if nc.partition_id_tensor else None)
    in_names, out_names, out_avals, zero_outs = [], [], [], []
    for alloc in nc.m.functions[0].allocations:
        if not isinstance(alloc, mybir.MemoryLocationSet):
            continue
        name = alloc.memorylocations[0].name
        if alloc.kind == "ExternalInput":
            if name != partition_name:
                in_names.append(name)
        elif alloc.kind == "ExternalOutput":
            shape = tuple(alloc.tensor_shape)
            dtype = mybir.dt.np(alloc.dtype)
            out_names.append(name)
            out_avals.append(jax.core.ShapedArray(shape, dtype))
            zero_outs.append(_np.zeros(shape, dtype))
    n_params = len(in_names)
    n_outs = len(out_avals)
    in_names_all = list(in_names) + list(out_names)
    if partition_name is not None:
        in_names_all.append(partition_name)

    def _body(*args):
        operands = list(args)
        if partition_name is not None:
            operands.append(partition_id_tensor())
        outs = _bass_exec_p.bind(
            *operands,
            out_avals=tuple(out_avals),
            in_names=tuple(in_names_all),
            out_names=tuple(out_names),
            lowering_input_output_aliases=(),
            sim_require_finite=True,
            sim_require_nnan=True,
            nc=nc,
        )
        return tuple(outs)

    devices = jax.devices()[:N_CORES]
    mesh = Mesh(np.asarray(devices), ("core",))
    in_specs = (PartitionSpec("core"),) * (n_params + n_outs)
    out_specs = (PartitionSpec("core"),) * n_outs
    donate = tuple(range(n_params, n_params + n_outs))
    sharded = jax.jit(
        shard_map(_body, mesh=mesh, in_specs=in_specs, out_specs=out_specs,
                  check_rep=False),
        donate_argnums=donate, keep_unused=True,
    )

    runner = {
        "sharded": sharded,
        "in_names": in_names,
        "out_names": out_names,
        "out_avals": out_avals,
        "zero_outs": zero_outs,
        "nc": nc,
    }
    _CACHE[key] = runner
    return runner


def _host_prep(q, k, v):
    """Shard + transform inputs for all cores. Returns dict name->global
    (n_cores*dim0, ...) arrays for shard_map."""
    bf16 = _np_bf16()
    qf = np.ascontiguousarray(q.reshape(BS * H, S, DK).transpose(0, 2, 1),
                              dtype=np.float32)
    kf = np.ascontiguousarray(k.reshape(BS * H, S, DK).transpose(0, 2, 1),
                              dtype=np.float32)
    qh = qf.astype(bf16)
    ql = (qf - qh.astype(np.float32)).astype(bf16)
    kh = kf.astype(bf16)
    kl = (kf - kh.astype(np.float32)).astype(bf16)
    qk4 = np.stack([qh, ql, kh, kl], axis=1)  # [BH*BS, 4, P, S]
    vb = v.reshape(BS * H, S, DK).astype(bf16)

    tri = np.where(np.arange(P)[:, None] > np.arange(P)[None, :],
                   np.float32(-1e38), np.float32(0.0)).astype(bf16)
    ident = np.eye(P, dtype=np.float32).astype(bf16)

    # global arrays: concat per-core shards along axis 0.
    # core c handles bh-flat rows [c*BH : (c+1)*BH] already (batch-major).
    glob = {
        "qk4": qk4, "v": vb,
        "tri": np.concatenate([tri[None]] * N_CORES, 0).reshape(
            N_CORES * P, P),
        "ident": np.concatenate([ident[None]] * N_CORES, 0).reshape(
            N_CORES * P, P),
    }
    return glob


def _host_rows(q, k, v, k_index):
    """Exact reference math for rows 0..k_index-1 (row 0 is zero-padded)."""
    qq = q[:, :, :k_index, :].astype(np.float64)
    kk = k.astype(np.float64)
    vv = v.astype(np.float64)
    s = np.einsum("bhqd,bhkd->bhqk", qq, kk) / np.sqrt(DK)
    j = np.arange(S)[None, None, None, :]
    i = np.arange(k_index)[None, None, :, None]
    s = np.where(j > i, NEG, s)
    p = np.exp(s - s.max(-1, keepdims=True))
    p = p / p.sum(-1, keepdims=True)
    w = np.exp(p)
    w = w / w.sum(-1, keepdims=True)
    out = np.einsum("bhqk,bhkd->bhqd", w, vv).astype(np.float32)
    out[:, :, 0, :] = 0.0
    return out


def _fallback(q, k, v, mask, k_index):
    """Pure-numpy replica of the reference (arbitrary mask / k_index)."""
    q64, k64, v64 = (x.astype(np.float64) for x in (q, k, v))
    s = np.einsum("bhqd,bhkd->bhqk", q64, k64) / np.sqrt(DK)
    s = np.where(np.asarray(mask) == 0, NEG, s)
    p = np.exp(s - s.max(-1, keepdims=True))
    p = p / p.sum(-1, keepdims=True)
    pa = p[:, :, :k_index, :]
    pb = p[:, :, k_index:, :]
    kth = -np.sort(-pb, axis=-1)[..., k_index - 1:k_index]
    pb = np.where(pb - kth >= 0, pb, NEG)
    sc = np.concatenate([pa, pb], axis=2)
    w = np.exp(sc - sc.max(-1, keepdims=True))
    w = w / w.sum(-1, keepdims=True)
    w[:, :, 0, :] = 0.0
    out = np.einsum("bhqk,bhkd->bhqd", w, v64)
    return out.astype(np.float32)


def kernel(q, k, v, mask, k_index):
    import birfix
    birfix.install()

    q = np.asarray(q, dtype=np.float32)
    k = np.asarray(k, dtype=np.float32)
    v = np.asarray(v, dtype=np.float32)
    ki = int(np.asarray(k_index))

    mask_np = np.asarray(mask)
    tril_ok = bool(
        (mask_np.reshape(S, S) == np.tril(np.ones((S, S), mask_np.dtype))).all()
    )
    if not tril_ok or not (1 <= ki <= 8):
        return _fallback(q, k, v, mask, ki)

    runner = _get_runner(ki, BH)
    glob = _host_prep(q, k, v)
    args = [glob[n] for n in runner["in_names"]]
    zeros = [np.zeros((N_CORES * z.shape[0], *z.shape[1:]), z.dtype)
             for z in runner["zero_outs"]]
    outs = runner["sharded"](*args, *zeros)
    out = np.array(outs[runner["out_names"].index("out")])
    out = out.reshape(BS, H, S, DK)
    out[:, :, :ki, :] = _host_rows(q, k, v, ki)
    return out


# birfix must be importable when kernel.py is standalone: embed a copy.
try:
    import birfix  # noqa: F401
except ImportError:
    import types

    _birfix_src = '''
import json
LIMIT = 1
_PATCHED = False

def split_waits_json(bir_json):
    d = json.loads(bir_json)
    cnt = 0
    for f in d.get("functions", []):
        for b in f.get("blocks", []):
            il = b.get("instructions")
            if not il:
                continue
            out = []
            changed = False
            for i in il:
                si = i.get("sync_info")
                waits = (si or {}).get("on_wait") or []
                if len(waits) > LIMIT:
                    changed = True
                    head, rest = waits[:-LIMIT], waits[-LIMIT:]
                    for ci in range(0, len(head), LIMIT):
                        cnt += 1
                        out.append({
                            "debug": i.get("debug", 0),
                            "engine": i["engine"],
                            "ins": [],
                            "is_reset_sema": False,
                            "name": "I-wsplit-%d" % cnt,
                            "opcode": "Drain",
                            "outs": [],
                            "sync_info": {"on_update": [],
                                          "on_wait": head[ci:ci + LIMIT]},
                        })
                    si["on_wait"] = rest
                out.append(i)
            if changed:
                b["instructions"] = out
    return json.dumps(d).encode()

def install():
    global _PATCHED
    if _PATCHED:
        return
    import concourse.bass2jax as b2j
    import concourse.bass_utils as bu
    orig = bu.compile_bir_kernel
    def patched(bir_json, tmpdir, neff_name="file.neff"):
        return orig(split_waits_json(bir_json), tmpdir, neff_name=neff_name)
    b2j.compile_bir_kernel = patched
    bu.compile_bir_kernel = patched
    _PATCHED = True
'''
    birfix = types.ModuleType("birfix")
    exec(_birfix_src, birfix.__dict__)
    sys.modules["birfix"] = birfix

sys.modules.setdefault("birfix_inline", sys.modules.get("birfix"))


# revision 30
# speedup vs baseline: 1.3541x; 1.0732x over previous
"""Trainium2 8-core kernel for nn_BAKT_QIKT (sparse causal attention with
top-k re-softmax).

Algorithm (validated numerically against the jax reference):
  scores s = q@k^T/sqrt(dk) with causal mask, p = softmax(s),
  rows >= k_index keep only top-k entries of p, re-softmax everything
  (masked entries contribute exp(0)=1 for rows < k_index), row 0 zeroed,
  out = w @ v.

Device mapping per (batch,head) pair (16 per core, data-parallel over batch):
  - scores in ~f32 precision via 3-term bf16-split matmul
    (qh@kh + ql@kh + qh@kl), causal mask added by an extra PE matmul
    (lhsT = strict-lower-tri * -1e38, rhs = identity) accumulating into the
    score PSUM.
  - e = exp(c*s) on ACT with row-sum accumulator Z (softmax denominators).
  - top-8 per row via DVE max8; kth = (k_index)-th largest of e.
  - second softmax shift-invariance: w~ = exp((e-kth)/Z) so kept entries
    are >= 1 and dropped < 1.  w2 = max(w~,1)-1 (kept-only, exact zeros),
    ind = (e >= kth) * 1/Z2 on GPSIMD; w2' = w2 * 1/Z2 on DVE.
    Z2 = sum(w2) + k_index (exactly k entries kept).
  - both layers PE-transposed (bf16) into separate PSUM tiles, merged by the
    mandatory PSUM->SBUF copy as tensor_tensor(add), then a single AV matmul
    chain accumulates out = (w2'+ind')^T-contracted with v into PSUM and is
    DMA'd straight to DRAM.
  Rows < k_index (and the zero-padded row 0) are computed on host - 0.5% of
  the work, removes all special cases from the device kernel.
"""
import os
import sys

sys.path.insert(0, "/opt/trn_rl_repo")

import numpy as np

BS, H, S, DK = 16, 8, 1024, 128
N_CORES = 8
B_PER_CORE = BS // N_CORES
BH = B_PER_CORE * H          # bh pairs per core
P = 128
NT = S // P                  # row tiles per bh
NEG = -1e32                  # reference's mask value
C_SCALE = 1.0 / np.sqrt(DK)

_CACHE = {}


def _np_bf16():
    import concourse.mybir as mybir
    return mybir.dt.np(mybir.dt.bfloat16)


def _build_nc(k_index: int, n_bh: int):
    import concourse.bass as bass
    import concourse.mybir as mybir
    from concourse.tile import TileContext

    BF = mybir.dt.bfloat16
    F32 = mybir.dt.float32
    AF = mybir.ActivationFunctionType
    OP = mybir.AluOpType

    debug = bool(os.environ.get("BAKT_DEBUG"))
    nc = bass.Bass()
    DK1 = DK + 1  # v carries a ones-column so AV also yields Z2 = sum(wsum)
    qk4_ext = nc.declare_dram_parameter("qk4", [n_bh, 4, P, S], BF,
                                        isOutput=False)
    v_ext = nc.declare_dram_parameter("v", [n_bh, S, DK1], BF, isOutput=False)
    tri_ext = nc.declare_dram_parameter("tri", [P, P], BF, isOutput=False)
    id_ext = nc.declare_dram_parameter("ident", [P, P], BF, isOutput=False)
    out_ext = nc.declare_dram_parameter("out", [n_bh, S, DK], F32, isOutput=True)
    if debug:
        e_dbg = nc.declare_dram_parameter("e_dbg", [P, 4608], F32, isOutput=True)
        m8_dbg = nc.declare_dram_parameter("m8_dbg", [P, NT, 8], F32, isOutput=True)
        zp_dbg = nc.declare_dram_parameter("zp_dbg", [P, NT], F32, isOutput=True)
        w2_dbg = nc.declare_dram_parameter("w2_dbg", [P, 4608], F32, isOutput=True)
        ws_dbg = nc.declare_dram_parameter("ws_dbg", [P, S], F32, isOutput=True)
        wt_dbg = nc.declare_dram_parameter("wt_dbg", [P, S], F32, isOutput=True)
        z2_dbg = nc.declare_dram_parameter("z2_dbg", [P, NT], F32, isOutput=True)

    with TileContext(nc) as tc:
        with (
            tc.tile_pool(name="consts", bufs=1) as cp,
            tc.tile_pool(name="qk", bufs=2) as qkp,
            tc.tile_pool(name="vp", bufs=2) as vp,
            tc.tile_pool(name="ep", bufs=2) as ep,
            tc.tile_pool(name="wp", bufs=2) as wp,
            tc.tile_pool(name="small", bufs=2) as sp,
            tc.tile_pool(name="ps_s", bufs=2, space="PSUM") as ps_s,
            tc.tile_pool(name="ps_t", bufs=2, space="PSUM") as ps_t,
            tc.tile_pool(name="ps_o", bufs=2, space="PSUM") as ps_o,
        ):
            tri = cp.tile([P, P], BF, tag="tri")
            nc.sync.dma_start(out=tri[:], in_=tri_ext[:])
            ident = cp.tile([P, P], BF, tag="ident")
            nc.sync.dma_start(out=ident[:], in_=id_ext[:])

            for bh in range(n_bh):
                qk4 = qkp.tile([P, 4, S], BF, tag="qk4")
                nc.sync.dma_start(
                    out=qk4[:], in_=qk4_ext[bh].rearrange("f p s -> p f s"))
                qh = qk4[:, 0, :]
                ql = qk4[:, 1, :]
                kh = qk4[:, 2, :]
                kl = qk4[:, 3, :]
                v = vp.tile([P, NT, DK1], BF, tag="v")
                nc.sync.dma_start(
                    out=v[:], in_=v_ext[bh].rearrange("(c p) d -> p c d", p=P)
                )

                e = ep.tile([P, 4608], F32, tag="e")
                m8p = sp.tile([P, NT, 8], F32, tag="m8p")
                zp = sp.tile([P, NT], F32, tag="zp")

                offs = [64 * t * (t + 1) for t in range(NT + 1)]

                # ---- phase A: scores, exp, row stats ----
                for t in range(NT):
                    C = P * (t + 1)
                    s_ps = ps_s.tile([P, S], F32, tag="s", space="PSUM")
                    qslice_h = qh[:, t * P:(t + 1) * P]
                    qslice_l = ql[:, t * P:(t + 1) * P]
                    for j0 in range(0, C, 512):
                        j1 = min(j0 + 512, C)
                        last = j1 == C
                        nc.tensor.matmul(s_ps[:, j0:j1], qslice_h, kh[:, j0:j1],
                                         start=True, stop=False)
                        nc.tensor.matmul(s_ps[:, j0:j1], qslice_l, kh[:, j0:j1],
                                         start=False, stop=False)
                        nc.tensor.matmul(s_ps[:, j0:j1], qslice_h, kl[:, j0:j1],
                                         start=False, stop=not last,
                                         skip_group_check=True)
                    # causal mask on the diagonal block via PE accumulation
                    nc.tensor.matmul(s_ps[:, t * P:C], tri[:], ident[:],
                                     start=False, stop=True,
                                     skip_group_check=True)
                    esl = e[:, offs[t]:offs[t] + C]
                    nc.scalar.activation(out=esl, in_=s_ps[:, :C], func=AF.Exp,
                                         scale=C_SCALE,
                                         accum_out=zp[:, t:t + 1])
                    nc.vector.max(out=m8p[:, t, :], in_=esl)

                # ---- packed per-bh stats ----
                rz = sp.tile([P, NT], F32, tag="rz")
                nc.vector.reciprocal(rz[:], zp[:])
                nkz = sp.tile([P, NT], F32, tag="nkz")
                nc.vector.tensor_tensor(out=nkz[:], in0=m8p[:, :, k_index - 1],
                                        in1=rz[:], op=OP.mult)
                nc.vector.tensor_scalar(nkz[:], nkz[:], -1.0, None, op0=OP.mult)

                # ---- phase B: w~ = exp((e-kth)/Z), wmax = max(w~,1) ----
                wtb = wp.tile([P, 4608], BF, tag="wtb")
                wmax = wp.tile([P, 4608], BF, tag="wmax")
                for t in range(NT):
                    C = P * (t + 1)
                    esl = e[:, offs[t]:offs[t] + C]
                    nc.scalar.activation(out=wtb[:, offs[t]:offs[t] + C],
                                         in_=esl, func=AF.Exp,
                                         scale=rz[:, t:t + 1],
                                         bias=nkz[:, t:t + 1])
                    nc.vector.tensor_scalar(wmax[:, offs[t]:offs[t] + C],
                                            wtb[:, offs[t]:offs[t] + C],
                                            1.0, None, op0=OP.max)

                if debug and bh == 0:
                    nc.sync.dma_start(out=e_dbg[:], in_=e[:])
                    nc.sync.dma_start(out=m8_dbg[:], in_=m8p[:])
                    nc.sync.dma_start(out=zp_dbg[:], in_=zp[:])
                    w2f = ep.tile([P, 4608], F32, tag="w2f")
                    nc.vector.tensor_copy(w2f[:], wmax[:])
                    nc.sync.dma_start(out=w2_dbg[:], in_=w2f[:])

                # ---- phase C: masked weights, Z2, transpose, AV ----
                for t in range(NT):
                    C = P * (t + 1)
                    esl = e[:, offs[t]:offs[t] + C]
                    # ind' = (e >= kth) - 1  in {0, -1}
                    ind = wp.tile([P, S], BF, tag="ind")
                    nc.gpsimd.tensor_scalar(ind[:, :C], esl,
                                            m8p[:, t, k_index - 1:k_index],
                                            -1.0, op0=OP.is_ge, op1=OP.add)
                    # wsum = wmax + ind' = {w~ kept, 0 dropped}
                    wsum = wp.tile([P, S], BF, tag="wsum")
                    nc.vector.tensor_tensor(out=wsum[:, :C],
                                            in0=wmax[:, offs[t]:offs[t] + C],
                                            in1=ind[:, :C], op=OP.add)
                    wt_ps = ps_t.tile([P, S], BF, tag="wt", space="PSUM")
                    for c in range(t + 1):
                        csl = slice(c * P, (c + 1) * P)
                        nc.tensor.matmul(wt_ps[:, csl], wsum[:, csl], ident[:],
                                         is_transpose=True)
                    wt_sb = wp.tile([P, S], BF, tag="wt_sb")
                    nc.vector.tensor_copy(wt_sb[:, :C], wt_ps[:, :C])
                    if debug and bh == 0 and t == 2:
                        wsf = wp.tile([P, S], F32, tag="wsf")
                        nc.vector.tensor_copy(wsf[:, :C], wsum[:, :C])
                        nc.sync.dma_start(out=ws_dbg[:, :C], in_=wsf[:, :C])
                        wtf = wp.tile([P, S], F32, tag="wtf")
                        nc.vector.tensor_copy(wtf[:, :C], wt_sb[:, :C])
                        nc.sync.dma_start(out=wt_dbg[:, :C], in_=wtf[:, :C])
                    if t % 3 == 0:
                        av_ps = ps_o.tile([P, 3 * DK1], F32, tag="av",
                                          space="PSUM")
                    tt = t % 3
                    for c in range(t + 1):
                        csl = slice(c * P, (c + 1) * P)
                        nc.tensor.matmul(av_ps[:, tt * DK1:(tt + 1) * DK1],
                                         wt_sb[:, csl], v[:, c, :],
                                         start=(c == 0), stop=(c == t))
                    if t % 3 == 2 or t == NT - 1:
                        g = tt + 1          # tiles in this group
                        t0 = t - tt
                        avv = av_ps[:].rearrange("p (g d) -> p g d", d=DK1)
                        rz3 = sp.tile([P, 3], F32, tag="rz3")
                        nc.vector.reciprocal(rz3[:, :g], avv[:, :g, DK])
                        outg = wp.tile([P, 3, DK], F32, tag="outg")
                        nc.vector.tensor_tensor(
                            out=outg[:, :g, :],
                            in0=avv[:, :g, :DK],
                            in1=rz3[:, :g, None].to_broadcast([P, g, DK]),
                            op=OP.mult)
                        nc.sync.dma_start(
                            out=out_ext[bh, t0 * P:(t0 + g) * P, :].rearrange(
                                "(c p) d -> p c d", p=P),
                            in_=outg[:, :g, :])
    return nc


def _get_runner(k_index: int, n_bh: int):
    """Build + jit once; reuse across calls (compile is minutes)."""
    key = (k_index, n_bh)
    if key in _CACHE:
        return _CACHE[key]

    import birfix_inline  # noqa: F401  (installed below; kept for clarity)

    nc = _build_nc(k_index, n_bh)

    import jax
    import numpy as _np
    from jax.sharding import Mesh, PartitionSpec
    from jax.experimental.shard_map import shard_map
    import concourse.mybir as mybir
    from concourse import bass2jax
    from concourse.bass2jax import _bass_exec_p, partition_id_tensor

    bass2jax.install_neuronx_cc_hook()

    partition_name = (nc.partition_id_tensor.name
                      if nc.partition_id_tensor else None)
    in_names, out_names, out_avals, zero_outs = [], [], [], []
    for alloc in nc.m.functions[0].allocations:
        if not isinstance(alloc, mybir.MemoryLocationSet):
            continue
        name = alloc.memorylocations[0].name
        if alloc.kind == "ExternalInput":
            if name != partition_name:
                in_names.append(name)
        elif alloc.kind == "ExternalOutput":
            shape = tuple(alloc.tensor_shape)
            dtype = mybir.dt.np(alloc.dtype)
            out_names.append(name)
            out_avals.append(jax.core.ShapedArray(shape, dtype))
            zero_outs.append(_np.zeros(shape, dtype))
    n_params = len(in_names)
    n_outs = len(out_avals)
    in_names_all = list(in_names) + list(out_names)
    if partition_name is not None:
        in_names_all.append(partition_name)

    def _body(*args):
        operands = list(args)
        if partition_name is not None:
            operands.append(partition_id_tensor())
        outs = _bass_exec_p.bind(
            *operands,
            out_avals=tuple(out_avals),
            in_names=tuple(in_names_all),
            out_names=tuple(out_names),
            lowering_input_output_aliases=(),
            sim_require_finite=True,
            sim_require_nnan=True,
            nc=nc,
        )
        return tuple(outs)

    devices = jax.devices()[:N_CORES]
    mesh = Mesh(np.asarray(devices), ("core",))
    in_specs = (PartitionSpec("core"),) * (n_params + n_outs)
    out_specs = (PartitionSpec("core"),) * n_outs
    donate = tuple(range(n_params, n_params + n_outs))
    sharded = jax.jit(
        shard_map(_body, mesh=mesh, in_specs=in_specs, out_specs=out_specs,
                  check_rep=False),
        donate_argnums=donate, keep_unused=True,
    )

    runner = {
        "sharded": sharded,
        "in_names": in_names,
        "out_names": out_names,
        "out_avals": out_avals,
        "zero_outs": zero_outs,
        "nc": nc,
    }
    _CACHE[key] = runner
    return runner


def _host_prep(q, k, v):
    """Shard + transform inputs for all cores. Returns dict name->global
    (n_cores*dim0, ...) arrays for shard_map."""
    bf16 = _np_bf16()
    qf = np.ascontiguousarray(q.reshape(BS * H, S, DK).transpose(0, 2, 1),
                              dtype=np.float32)
    kf = np.ascontiguousarray(k.reshape(BS * H, S, DK).transpose(0, 2, 1),
                              dtype=np.float32)
    qh = qf.astype(bf16)
    ql = (qf - qh.astype(np.float32)).astype(bf16)
    kh = kf.astype(bf16)
    kl = (kf - kh.astype(np.float32)).astype(bf16)
    qk4 = np.stack([qh, ql, kh, kl], axis=1)  # [BH*BS, 4, P, S]
    vb = np.concatenate(
        [v.reshape(BS * H, S, DK),
         np.ones((BS * H, S, 1), np.float32)], axis=-1).astype(bf16)

    tri = np.where(np.arange(P)[:, None] > np.arange(P)[None, :],
                   np.float32(-1e38), np.float32(0.0)).astype(bf16)
    ident = np.eye(P, dtype=np.float32).astype(bf16)

    # global arrays: concat per-core shards along axis 0.
    # core c handles bh-flat rows [c*BH : (c+1)*BH] already (batch-major).
    glob = {
        "qk4": qk4, "v": vb,
        "tri": np.concatenate([tri[None]] * N_CORES, 0).reshape(
            N_CORES * P, P),
        "ident": np.concatenate([ident[None]] * N_CORES, 0).reshape(
            N_CORES * P, P),
    }
    return glob


def _host_rows(q, k, v, k_index):
    """Exact reference math for rows 0..k_index-1 (row 0 is zero-padded)."""
    qq = q[:, :, :k_index, :].astype(np.float64)
    kk = k.astype(np.float64)
    vv = v.astype(np.float64)
    s = np.einsum("bhqd,bhkd->bhqk", qq, kk) / np.sqrt(DK)
    j = np.arange(S)[None, None, None, :]
    i = np.arange(k_index)[None, None, :, None]
    s = np.where(j > i, NEG, s)
    p = np.exp(s - s.max(-1, keepdims=True))
    p = p / p.sum(-1, keepdims=True)
    w = np.exp(p)
    w = w / w.sum(-1, keepdims=True)
    out = np.einsum("bhqk,bhkd->bhqd", w, vv).astype(np.float32)
    out[:, :, 0, :] = 0.0
    return out


def _fallback(q, k, v, mask, k_index):
    """Pure-numpy replica of the reference (arbitrary mask / k_index)."""
    q64, k64, v64 = (x.astype(np.float64) for x in (q, k, v))
    s = np.einsum("bhqd,bhkd->bhqk", q64, k64) / np.sqrt(DK)
    s = np.where(np.asarray(mask) == 0, NEG, s)
    p = np.exp(s - s.max(-1, keepdims=True))
    p = p / p.sum(-1, keepdims=True)
    pa = p[:, :, :k_index, :]
    pb = p[:, :, k_index:, :]
    kth = -np.sort(-pb, axis=-1)[..., k_index - 1:k_index]
    pb = np.where(pb - kth >= 0, pb, NEG)
    sc = np.concatenate([pa, pb], axis=2)
    w = np.exp(sc - sc.max(-1, keepdims=True))
    w = w / w.sum(-1, keepdims=True)
    w[:, :, 0, :] = 0.0
    out = np.einsum("bhqk,bhkd->bhqd", w, v64)
    return out.astype(np.float32)


def kernel(q, k, v, mask, k_index):
    import birfix
    birfix.install()

    q = np.asarray(q, dtype=np.float32)
    k = np.asarray(k, dtype=np.float32)
    v = np.asarray(v, dtype=np.float32)
    ki = int(np.asarray(k_index))

    mask_np = np.asarray(mask)
    tril_ok = bool(
        (mask_np.reshape(S, S) == np.tril(np.ones((S, S), mask_np.dtype))).all()
    )
    if not tril_ok or not (1 <= ki <= 8):
        return _fallback(q, k, v, mask, ki)

    runner = _get_runner(ki, BH)
    glob = _host_prep(q, k, v)
    args = [glob[n] for n in runner["in_names"]]
    zeros = [np.zeros((N_CORES * z.shape[0], *z.shape[1:]), z.dtype)
             for z in runner["zero_outs"]]
    outs = runner["sharded"](*args, *zeros)
    out = np.array(outs[runner["out_names"].index("out")])
    out = out.reshape(BS, H, S, DK)
    out[:, :, :ki, :] = _host_rows(q, k, v, ki)
    return out


# birfix must be importable when kernel.py is standalone: embed a copy.
try:
    import birfix  # noqa: F401
except ImportError:
    import types

    _birfix_src = '''
import json
LIMIT = 1
_PATCHED = False

def split_waits_json(bir_json):
    d = json.loads(bir_json)
    cnt = 0
    for f in d.get("functions", []):
        for b in f.get("blocks", []):
            il = b.get("instructions")
            if not il:
                continue
            out = []
            changed = False
            for i in il:
                si = i.get("sync_info")
                waits = (si or {}).get("on_wait") or []
                if len(waits) > LIMIT:
                    changed = True
                    head, rest = waits[:-LIMIT], waits[-LIMIT:]
                    for ci in range(0, len(head), LIMIT):
                        cnt += 1
                        out.append({
                            "debug": i.get("debug", 0),
                            "engine": i["engine"],
                            "ins": [],
                            "is_reset_sema": False,
                            "name": "I-wsplit-%d" % cnt,
                            "opcode": "Drain",
                            "outs": [],
                            "sync_info": {"on_update": [],
                                          "on_wait": head[ci:ci + LIMIT]},
                        })
                    si["on_wait"] = rest
                out.append(i)
            if changed:
                b["instructions"] = out
    return json.dumps(d).encode()

def install():
    global _PATCHED
    if _PATCHED:
        return
    import concourse.bass2jax as b2j
    import concourse.bass_utils as bu
    orig = bu.compile_bir_kernel
    def patched(bir_json, tmpdir, neff_name="file.neff"):
        return orig(split_waits_json(bir_json), tmpdir, neff_name=neff_name)
    b2j.compile_bir_kernel = patched
    bu.compile_bir_kernel = patched
    _PATCHED = True
'''
    birfix = types.ModuleType("birfix")
    exec(_birfix_src, birfix.__dict__)
    sys.modules["birfix"] = birfix

sys.modules.setdefault("birfix_inline", sys.modules.get("birfix"))
